# revision 1
# baseline (speedup 1.0000x reference)
"""GAT layer (nn_GAT_layer_67619965108552) as a Trainium2 Bass/Tile SPMD kernel.

Structure exploited (verified vs reference to 1.6e-4 in bf16):
  With n=8192, the buggy-but-faithful pair indexing collapses:
    rows i < 4096:  scores[i, j] = u[2i + (j >= 4096)],  u = x @ (W@a1 + W@a2)
    rows i >= 4096: scores[i, j] = tt[j mod 4096],       tt = xe@(W@a1) + xo@(W@a2)
  (xe/xo = even/odd rows of x). After leaky_relu + adj masking + softmax,
  attn @ out reduces to two masked row-sum matmuls against adj halves:
    Y1 = A[:, :4096] @ [f*out_L | f],  Y2 = A[:, 4096:] @ [f*out_R | f]
    res = sigmoid((al1*Y1 + al2*Y2)[:, :256] / (al1*Y1 + al2*Y2)[:, 256])
  Top-half cores use f = 1, al1 = exp(lrelu(u_even)), al2 = exp(lrelu(u_odd));
  bottom-half cores use f = exp(lrelu(tt)), al1 = al2 = 1. Both variants run the
  identical instruction stream; only input data (g / select masks) differs.

Sharding: rows of adj (and of the output) across 8 cores, 1024 rows each.
x/weight/att_vec replicated; every core computes the full out = x@W (needed as
the rhs of its adj matmul).
"""
import numpy as np
from contextlib import ExitStack

import concourse.bass as bass
import concourse.tile as tile
from concourse import bacc, mybir
from concourse.masks import make_identity
from concourse.bass_utils import run_bass_kernel_spmd

F32 = mybir.dt.float32
BF16 = mybir.dt.bfloat16
I32 = mybir.dt.int32

N = 8192          # nodes
FIN = 512         # input features
FOUT = 256        # output features
P = 128
NB = N // P       # 64 i-blocks over all nodes
NCORES = 8
RPC = N // NCORES  # 1024 rows per core
MB = RPC // P      # 8 output row-blocks per core
HKC = 32           # j-chunks per half (4096/128)
GJC = 16           # j-chunks per stage-B group
NGRP = NB // GJC   # 4 groups


def build_program():
    nc = bacc.Bacc("TRN2", target_bir_lowering=False, debug=False,
                   num_devices=NCORES)

    x_d = nc.dram_tensor("x", [N, FIN], F32, kind="ExternalInput")
    w_d = nc.dram_tensor("w", [FIN, FOUT], F32, kind="ExternalInput")
    attb_d = nc.dram_tensor("attb", [P, 2 * FOUT], F32, kind="ExternalInput")
    adj_d = nc.dram_tensor("adj", [RPC, N], I32, kind="ExternalInput")
    # gcol[:, 0] = g (1.0 for top-half cores, 0.0 for bottom), gcol[:, 1] = 1-g
    g_d = nc.dram_tensor("gcol", [P, 2], F32, kind="ExternalInput")
    # selg[p, B', B] = g * (B == 8c + B') : per-core row-block select
    selg_d = nc.dram_tensor("selg", [P, MB, HKC], F32, kind="ExternalInput")
    # basis vectors for psum row extraction: col 0 -> row 1, col 1 -> row 2
    eb_d = nc.dram_tensor("ebasis", [3, 2], F32, kind="ExternalInput")
    y_d = nc.dram_tensor("y", [RPC, FOUT], F32, kind="ExternalOutput")

    with tile.TileContext(nc) as tc, ExitStack() as ctx:
        constp = ctx.enter_context(tc.tile_pool(name="const", bufs=1))
        dramp = ctx.enter_context(tc.tile_pool(name="dram", bufs=1, space="DRAM"))

        # ---- constants ----
        ident = constp.tile([P, P], BF16)
        make_identity(nc, ident)
        wtile = constp.tile([P, 4, FOUT], F32)     # W, k-chunk major
        nc.sync.dma_start(wtile[:], w_d.ap().rearrange("(c p) f -> p c f", p=P))
        wbf = constp.tile([P, 4, FOUT], BF16)
        nc.vector.tensor_copy(wbf[:], wtile[:])
        attb = constp.tile([P, 2 * FOUT], F32)     # [a1 | a2], partition-bcast
        nc.sync.dma_start(attb[:], attb_d.ap())
        gcol = constp.tile([P, 2], F32)
        nc.sync.dma_start(gcol[:], g_d.ap())
        selg = constp.tile([P, MB, HKC], F32)
        nc.sync.dma_start(selg[:], selg_d.ap())
        ebasis = constp.tile([3, 2], F32)
        nc.sync.dma_start(ebasis[:], eb_d.ap())

        # persistent mid-size tensors
        outb = [constp.tile([P, HKC, FOUT + 1], BF16, name=f"outb{h}")
                for h in range(2)]
        ysb = [constp.tile([P, MB, FOUT + 1], F32, name=f"ysb{h}")
               for h in range(2)]
        aepm = constp.tile([P, HKC], F32)
        bepm = constp.tile([P, HKC], F32)
        vpm = constp.tile([P, HKC], F32)
        fpm = constp.tile([P, HKC], F32)
        al1 = constp.tile([P, MB], F32)
        al2 = constp.tile([P, MB], F32)
        wamf = constp.tile([P, 4, 3], F32)
        wam = constp.tile([P, 4, 3], BF16)

        # ---- stage A (scoped pools) ----
        with tc.tile_pool(name="svp", bufs=1) as svp, \
             tc.tile_pool(name="sa", bufs=3) as sa, \
             tc.tile_pool(name="xtp", bufs=6) as xtp, \
             tc.tile_pool(name="ps_t", bufs=2, space="PSUM") as ps_t, \
             tc.tile_pool(name="ps_a", bufs=2, space="PSUM") as ps_a, \
             tc.tile_pool(name="ps_u", bufs=2, space="PSUM") as ps_u:

            # wam[:, c, :] = [wu | wa1 | wa2] chunk c (bf16 lhsT for U-matmul)
            for c in range(4):
                t = sa.tile([P, FOUT], F32, tag="wa_tmp", name="wa_tmp")
                nc.vector.tensor_mul(t[:], wtile[:, c, :], attb[:, :FOUT])
                nc.vector.tensor_reduce(wamf[:, c, 1:2], t[:],
                                        axis=mybir.AxisListType.X,
                                        op=mybir.AluOpType.add)
                t2 = sa.tile([P, FOUT], F32, tag="wa_tmp", name="wa_tmp2")
                nc.vector.tensor_mul(t2[:], wtile[:, c, :], attb[:, FOUT:])
                nc.vector.tensor_reduce(wamf[:, c, 2:3], t2[:],
                                        axis=mybir.AxisListType.X,
                                        op=mybir.AluOpType.add)
                nc.vector.tensor_add(wamf[:, c, 0:1], wamf[:, c, 1:2],
                                     wamf[:, c, 2:3])
            nc.vector.tensor_copy(wam[:], wamf[:])

            # svl_* accumulate the pre-activation score vectors (free-major,
            # partition 0 only — engines can't start reads at partitions 1/2)
            svl_ae = svp.tile([1, N // 2], F32)
            svl_be = svp.tile([1, N // 2], F32)
            svl_tt = svp.tile([1, N // 2], F32)

            for g4 in range(16):          # groups of 4 i-blocks
                xts = []
                for bi in range(4):
                    b = g4 * 4 + bi
                    xb = sa.tile([P, FIN], BF16, tag="xb", name="xb")
                    nc.gpsimd.dma_start(xb[:], x_d.ap()[b * P:(b + 1) * P, :])
                    xt = xtp.tile([P, 4, P], BF16, tag="xt", name="xt")
                    for c in range(4):
                        pst = ps_t.tile([P, P], BF16, tag="pstx", name="pstx")
                        nc.tensor.transpose(pst[:], xb[:, c * P:(c + 1) * P],
                                            ident[:])
                        if (b + c) % 2 == 0:
                            nc.vector.tensor_copy(xt[:, c, :], pst[:])
                        else:
                            nc.scalar.copy(xt[:, c, :], pst[:])
                    xts.append(xt)
                    po = ps_a.tile([P, FOUT], F32, tag="po", name="po")
                    for c in range(4):
                        nc.tensor.matmul(po[:], xt[:, c, :], wbf[:, c, :],
                                         start=(c == 0), stop=(c == 3))
                    h, kc = (0, b) if b < HKC else (1, b - HKC)
                    if b % 2 == 0:
                        nc.scalar.copy(outb[h][:, kc, :FOUT], po[:])
                    else:
                        nc.vector.tensor_copy(outb[h][:, kc, :FOUT], po[:])

                # U-matmul over the 4 blocks: pu4[:, bi] = [u; s1; s2]
                pu4 = ps_u.tile([3, 4 * P], F32, tag="pu4", name="pu4", bufs=1)
                for bi in range(4):
                    for c in range(4):
                        nc.tensor.matmul(pu4[:, bi * P:(bi + 1) * P],
                                         wam[:, c, :], xts[bi][:, c, :],
                                         start=(c == 0), stop=(c == 3))
                pusb = sa.tile([3, 4 * P], F32, tag="pusb", name="pusb")
                nc.vector.tensor_copy(pusb[:], pu4[:])
                # tt = s1[even] + s2[odd]: accumulate both basis extractions
                # (rows 1 and 2 of pusb, parity-sliced) into one PSUM tile
                ext = ps_u.tile([1, 2 * P], F32, tag="ext", name="ext", bufs=1)
                nc.tensor.matmul(ext[:], ebasis[:, 0:1], pusb[:, 0::2],
                                 start=True, stop=False)
                nc.tensor.matmul(ext[:], ebasis[:, 1:2], pusb[:, 1::2],
                                 start=False, stop=True)
                sl = slice(g4 * 256, (g4 + 1) * 256)
                nc.vector.tensor_copy(svl_ae[0:1, sl], pusb[0:1, 0::2])
                nc.vector.tensor_copy(svl_be[0:1, sl], pusb[0:1, 1::2])
                nc.vector.tensor_copy(svl_tt[0:1, sl], ext[:])

            # ---- small vectors: ae, be, v -> DRAM -> partition-major ----
            vec_dram = dramp.tile([3, N // 2], F32)
            for row, vec in ((0, svl_ae), (1, svl_be), (2, svl_tt)):
                svm = svp.tile([1, N // 2], F32, tag="svm", name="svm")
                nc.vector.tensor_scalar_mul(svm[:], vec[:], 0.01)
                nc.vector.tensor_max(svm[:], vec[:], svm[:])
                nc.scalar.activation(vec[:], svm[:],
                                     mybir.ActivationFunctionType.Exp)
                nc.gpsimd.dma_start(vec_dram[row:row + 1, :], vec[:])
            for row, dst in ((0, aepm), (1, bepm), (2, vpm)):
                nc.gpsimd.dma_start(
                    dst[:],
                    vec_dram[row:row + 1, :].rearrange("r (B p) -> (r p) B",
                                                       p=P))

            # f = g + (1-g)*v  (per-partition scalars from gcol)
            nc.vector.tensor_scalar(fpm[:], vpm[:], gcol[:, 1:2], gcol[:, 0:1],
                                    op0=mybir.AluOpType.mult,
                                    op1=mybir.AluOpType.add)
            # alphas: al{1,2}[:, B'] = sum_B {ae,be}pm[:, B]*selg[:, B', B] + 1-g
            for bp in range(MB):
                m1 = sa.tile([P, HKC], F32, tag="alm", name="alm1")
                nc.vector.tensor_mul(m1[:], aepm[:], selg[:, bp, :])
                nc.vector.tensor_reduce(al1[:, bp:bp + 1], m1[:],
                                        axis=mybir.AxisListType.X,
                                        op=mybir.AluOpType.add)
                m2 = sa.tile([P, HKC], F32, tag="alm", name="alm2")
                nc.vector.tensor_mul(m2[:], bepm[:], selg[:, bp, :])
                nc.vector.tensor_reduce(al2[:, bp:bp + 1], m2[:],
                                        axis=mybir.AxisListType.X,
                                        op=mybir.AluOpType.add)
            nc.vector.tensor_scalar_add(al1[:], al1[:], gcol[:, 1:2])
            nc.vector.tensor_scalar_add(al2[:], al2[:], gcol[:, 1:2])

            # rhs finalize: scale out rows by f, write f into column FOUT
            for h in range(2):
                nc.vector.tensor_copy(outb[h][:, :, FOUT:FOUT + 1], fpm[:])
            for b in range(NB):
                h, kc = (0, b) if b < HKC else (1, b - HKC)
                dst = outb[h][:, kc, :FOUT]
                if b % 2 == 0:
                    nc.vector.tensor_scalar_mul(dst, dst, fpm[:, kc:kc + 1])
                else:
                    nc.scalar.activation(dst, dst,
                                         mybir.ActivationFunctionType.Copy,
                                         scale=fpm[:, kc:kc + 1])

        # ---- stages B+C: adj -> AT groups -> Y partial accumulation ----
        with tc.tile_pool(name="astgp", bufs=6) as astgp, \
             tc.tile_pool(name="atgp", bufs=2) as atgp, \
             tc.tile_pool(name="comb", bufs=2) as comb, \
             tc.tile_pool(name="ps_tb", bufs=2, space="PSUM") as ps_tb, \
             tc.tile_pool(name="ps_y", bufs=2, space="PSUM") as ps_y:

            for grp in range(NGRP):
                h = grp // 2          # 0: left half (j<4096), 1: right half
                atg = atgp.tile([P, GJC, RPC], BF16, tag="atg", name="atg")
                for t in range(GJC):
                    jc = grp * GJC + t
                    asb = astgp.tile([P, MB, P], BF16, tag="asb", name="asb")
                    nc.gpsimd.dma_start(
                        asb[:],
                        adj_d.ap()[:, jc * P:(jc + 1) * P].rearrange(
                            "(ib p) j -> p ib j", p=P))
                    pst = ps_tb.tile([P, RPC], BF16, tag="pstA", name="pstA")
                    for ib in range(MB):
                        nc.tensor.transpose(pst[:, ib * P:(ib + 1) * P],
                                            asb[:, ib, :], ident[:])
                    nc.scalar.copy(atg[:, t, :], pst[:])
                for mb in range(MB):
                    yp = ps_y.tile([P, FOUT + 1], F32, tag="yp", name="yp")
                    for t in range(GJC):
                        kc = (grp % 2) * GJC + t
                        nc.tensor.matmul(yp[:],
                                         atg[:, t, mb * P:(mb + 1) * P],
                                         outb[h][:, kc, :],
                                         start=(t == 0), stop=(t == GJC - 1))
                    if grp % 2 == 0:
                        nc.vector.tensor_copy(ysb[h][:, mb, :], yp[:])
                    else:
                        nc.vector.tensor_add(ysb[h][:, mb, :],
                                             ysb[h][:, mb, :], yp[:])

            # ---- combine + sigmoid + store ----
            for mb in range(MB):
                z1 = comb.tile([P, FOUT + 1], F32, tag="z1", name="z1")
                z2 = comb.tile([P, FOUT + 1], F32, tag="z2", name="z2")
                nc.vector.tensor_scalar_mul(z1[:], ysb[0][:, mb, :],
                                            al1[:, mb:mb + 1])
                nc.vector.tensor_scalar_mul(z2[:], ysb[1][:, mb, :],
                                            al2[:, mb:mb + 1])
                nc.vector.tensor_add(z1[:], z1[:], z2[:])
                rec = comb.tile([P, 1], F32, tag="rec", name="rec")
                nc.vector.reciprocal(rec[:], z1[:, FOUT:FOUT + 1])
                res = comb.tile([P, FOUT], F32, tag="res", name="res")
                nc.vector.tensor_scalar_mul(res[:], z1[:, :FOUT], rec[:])
                resg = comb.tile([P, FOUT], F32, tag="resg", name="resg")
                nc.scalar.activation(resg[:], res[:],
                                     mybir.ActivationFunctionType.Sigmoid)
                nc.sync.dma_start(y_d.ap()[mb * P:(mb + 1) * P, :], resg[:])

    nc.compile()
    return nc


_NC_CACHE = None


def _get_program():
    global _NC_CACHE
    if _NC_CACHE is None:
        _NC_CACHE = build_program()
    return _NC_CACHE


def make_in_maps(x, weight, att_vec, adj):
    x = np.ascontiguousarray(np.asarray(x, dtype=np.float32))
    weight = np.ascontiguousarray(np.asarray(weight, dtype=np.float32))
    att_vec = np.asarray(att_vec, dtype=np.float32)
    adj = np.asarray(adj, dtype=np.int32)

    attb = np.broadcast_to(att_vec[:, 0][None, :], (P, 2 * FOUT)).copy()
    in_maps = []
    for c in range(NCORES):
        g = 1.0 if c < 4 else 0.0
        gcol = np.empty((P, 2), np.float32)
        gcol[:, 0] = g
        gcol[:, 1] = 1.0 - g
        selg = np.zeros((P, MB, HKC), np.float32)
        for bp in range(MB):
            selg[:, bp, (c * MB + bp) % HKC] = g
        ebasis = np.array([[0.0, 0.0], [1.0, 0.0], [0.0, 1.0]], np.float32)
        in_maps.append({
            "x": x,
            "w": weight,
            "attb": attb,
            "adj": np.ascontiguousarray(adj[c * RPC:(c + 1) * RPC, :]),
            "gcol": gcol,
            "selg": selg,
            "ebasis": ebasis,
        })
    return in_maps


def kernel(x, weight, att_vec, adj, _trace=False, _trace_kwargs=None):
    nc = _get_program()
    in_maps = make_in_maps(x, weight, att_vec, adj)
    r = run_bass_kernel_spmd(nc, in_maps, core_ids=list(range(NCORES)),
                             trace=_trace, **(_trace_kwargs or {}))
    y = np.concatenate([r.results[c]["y"] for c in range(NCORES)], axis=0)
    kernel.last_results = r
    return y.astype(np.float32)



# revision 6
# speedup vs baseline: 1.7039x; 1.7039x over previous
"""GAT layer (nn_GAT_layer_67619965108552) as a Trainium2 Bass/Tile SPMD kernel.

Structure exploited (validated vs reference to 5.2e-3 in bf16):
  With n=8192, the buggy-but-faithful pair indexing collapses:
    rows i < 4096:  scores[i, j] = u[2i + (j >= 4096)],  u = x @ (W@a1 + W@a2)
    rows i >= 4096: scores[i, j] = tt[j mod 4096],  tt[q] = s1[2q] + s2[2q+1]
  After leaky_relu + adj masking + softmax, attn @ out reduces to masked
  row-sum matmuls against f-scaled out:
    ysb[h][mb] = sum_{kc in half h} adjT(mb,kc)^T @ (f * [out | 1])[kc]
    res = sigmoid((al1*ysbL + al2*ysbR)[:, :256] / (...)[:, 256])
  Top-half cores: f = 1, al1/al2 = exp(lrelu(u at even/odd rows)); bottom-half
  cores: f = exp(lrelu(tt)), al = 1. One instruction stream for all cores;
  divergence is data-driven (g flag / select masks).

Single fused pipeline in 8 super-tiles: x^T streams on the HWDGE queue while
adj (int32 -> bf16 cast) streams on the SWDGE queue in 4MB column-group DMAs
with 4KB-contiguous runs; u/s1/s2 come free as 3 extra matmul columns; the
per-pair score vectors are extracted with constant 0/1 parity-pick matmuls
(no DRAM round trip); f-scaling is per-chunk so nothing serializes globally.

Sharding: rows of adj / output across 8 cores (1024 each); x/weight/att_vec
replicated (each core computes the full out = x@W as the rhs of its matmuls).
"""
import ml_dtypes
import numpy as np
from contextlib import ExitStack

import concourse.bass as bass
import concourse.tile as tile
from concourse import bacc, mybir
from concourse.masks import make_identity
from concourse.bass_utils import run_bass_kernel_spmd

F32 = mybir.dt.float32
BF16 = mybir.dt.bfloat16
I32 = mybir.dt.int32

N = 8192           # nodes
FIN = 512          # input features
FOUT = 256         # output features
P = 128
NB = N // P        # 64 row-blocks of out
NCORES = 8
RPC = N // NCORES  # 1024 rows per core
MB = RPC // P      # 8 output row-blocks per core
NST = 8            # super-tiles (adj column groups of 1024)


def build_program():
    nc = bacc.Bacc("TRN2", target_bir_lowering=False, debug=False,
                   num_devices=NCORES)

    xt_d = nc.dram_tensor("xt", [FIN, N], BF16, kind="ExternalInput")
    w_d = nc.dram_tensor("w", [FIN, FOUT], F32, kind="ExternalInput")
    attb_d = nc.dram_tensor("attb", [P, 2 * FOUT], F32, kind="ExternalInput")
    adj_d = nc.dram_tensor("adj", [RPC, N], I32, kind="ExternalInput")
    # gcol[:, 0] = g (1.0 top-half cores, 0.0 bottom), gcol[:, 1] = 1-g
    g_d = nc.dram_tensor("gcol", [P, 2], F32, kind="ExternalInput")
    # selg[p, mb, m] = g * (m == 8c + mb) : per-core pair select for alphas
    selg_d = nc.dram_tensor("selg", [P, MB, 32], F32, kind="ExternalInput")
    # parity-pick matrices [E0 | E1 | Eo0 | Eo1]
    emat_d = nc.dram_tensor("emat", [P, 4, P], BF16, kind="ExternalInput")
    y_d = nc.dram_tensor("y", [RPC, FOUT], F32, kind="ExternalOutput")

    Exp = mybir.ActivationFunctionType.Exp
    Sigmoid = mybir.ActivationFunctionType.Sigmoid
    AX = mybir.AxisListType.X
    ADD = mybir.AluOpType.add
    MULT = mybir.AluOpType.mult

    with tile.TileContext(nc) as tc, ExitStack() as ctx:
        constp = ctx.enter_context(tc.tile_pool(name="const", bufs=1))

        # ---- constants ----
        ident = constp.tile([P, P], BF16)
        make_identity(nc, ident)
        emat = constp.tile([P, 4, P], BF16)
        nc.sync.dma_start(emat[:], emat_d.ap())
        gcol = constp.tile([P, 2], F32)
        nc.sync.dma_start(gcol[:], g_d.ap())
        selg = constp.tile([P, MB, 32], F32)
        nc.sync.dma_start(selg[:], selg_d.ap())
        attb = constp.tile([P, 2 * FOUT], F32)
        nc.sync.dma_start(attb[:], attb_d.ap())
        wtile = constp.tile([P, 4, FOUT], F32)
        nc.sync.dma_start(wtile[:], w_d.ap().rearrange("(c p) f -> p c f", p=P))

        # persistent tensors
        wrhs = constp.tile([P, 4, FOUT + 3], BF16)   # [W | wu | w1 | w2]
        outb = constp.tile([P, NB, FOUT + 1], BF16)  # f*[out | 1] chunks
        aepm = constp.tile([P, 32], F32)
        bepm = constp.tile([P, 32], F32)
        fpm = constp.tile([P, 32], F32)
        al1 = constp.tile([P, MB], F32)
        al2 = constp.tile([P, MB], F32)
        ysb = [constp.tile([P, MB, FOUT + 1], F32, name=f"ysb{h}")
               for h in range(2)]

        with tc.tile_pool(name="sa", bufs=4) as sa, \
             tc.tile_pool(name="xtp", bufs=2) as xtp, \
             tc.tile_pool(name="adjp", bufs=2) as adjp, \
             tc.tile_pool(name="atgp", bufs=2) as atgp, \
             tc.tile_pool(name="ps_po", bufs=2, space="PSUM") as ps_po, \
             tc.tile_pool(name="ps_at", bufs=2, space="PSUM") as ps_at, \
             tc.tile_pool(name="ps_y", bufs=2, space="PSUM") as ps_y, \
             tc.tile_pool(name="ps_u", bufs=2, space="PSUM") as ps_u:

            # build wrhs = [W | W@(a1+a2) | W@a1 | W@a2] per k-chunk, bf16
            wamf = sa.tile([P, 4, 3], F32, tag="wamf", name="wamf")
            for c in range(4):
                t1 = sa.tile([P, FOUT], F32, tag="wa_tmp", name="wa1")
                nc.vector.tensor_mul(t1[:], wtile[:, c, :], attb[:, :FOUT])
                nc.vector.tensor_reduce(wamf[:, c, 1:2], t1[:], axis=AX, op=ADD)
                t2 = sa.tile([P, FOUT], F32, tag="wa_tmp", name="wa2")
                nc.vector.tensor_mul(t2[:], wtile[:, c, :], attb[:, FOUT:])
                nc.vector.tensor_reduce(wamf[:, c, 2:3], t2[:], axis=AX, op=ADD)
                nc.vector.tensor_add(wamf[:, c, 0:1], wamf[:, c, 1:2],
                                     wamf[:, c, 2:3])
                nc.vector.tensor_copy(wrhs[:, c, :FOUT], wtile[:, c, :])
                nc.vector.tensor_copy(wrhs[:, c, FOUT:], wamf[:, c, :])

            for g in range(NST):
                # ---- DMAs for this super-tile ----
                if g < 4:
                    xts = xtp.tile([P, 4, 16 * P], BF16, tag="xts", name="xts")
                    nc.sync.dma_start(
                        xts[:],
                        xt_d.ap()[:, g * 2048:(g + 1) * 2048].rearrange(
                            "(c p) r -> p c r", p=P))
                asb = adjp.tile([P, MB, NST * P], BF16, tag="asb", name="asb")
                nc.gpsimd.dma_start(
                    asb[:],
                    adj_d.ap()[:, g * 1024:(g + 1) * 1024].rearrange(
                        "(ib p) j -> p ib j", p=P))

                # ---- stage A: out chunks + u/s1/s2 for 16 x-blocks ----
                if g < 4:
                    usbE = sa.tile([P, 24], BF16, tag="usbE", name="usbE")
                    usbO = sa.tile([P, 24], BF16, tag="usbO", name="usbO")
                    for pair in range(8):
                        for half in range(2):
                            lb = 2 * pair + half      # local block 0..15
                            b = 16 * g + lb           # global block
                            po = ps_po.tile([P, FOUT + 3], F32, tag="po",
                                            name="po")
                            for c in range(4):
                                nc.tensor.matmul(
                                    po[:], xts[:, c, lb * P:(lb + 1) * P],
                                    wrhs[:, c, :],
                                    start=(c == 0), stop=(c == 3))
                            if b % 2 == 0:
                                nc.vector.tensor_copy(outb[:, b, :FOUT],
                                                      po[:, :FOUT])
                            else:
                                nc.scalar.copy(outb[:, b, :FOUT],
                                               po[:, :FOUT])
                            dst = usbE if half == 0 else usbO
                            nc.vector.tensor_copy(
                                dst[:, pair * 3:(pair + 1) * 3],
                                po[:, FOUT:FOUT + 3])
                    # parity-pick matmuls: even picks cols 0:24, odd 24:48
                    pv = ps_u.tile([P, 48], F32, tag="pv", name="pv")
                    nc.tensor.matmul(pv[:, 0:24], emat[:, 0, :], usbE[:],
                                     start=True, stop=False)
                    nc.tensor.matmul(pv[:, 0:24], emat[:, 1, :], usbO[:],
                                     start=False, stop=True)
                    nc.tensor.matmul(pv[:, 24:48], emat[:, 2, :], usbE[:],
                                     start=True, stop=False)
                    nc.tensor.matmul(pv[:, 24:48], emat[:, 3, :], usbO[:],
                                     start=False, stop=True)
                    pvs = sa.tile([P, 48], F32, tag="pvs", name="pvs")
                    nc.vector.tensor_copy(pvs[:], pv[:])
                    # abv = [ae_pre(8) | be_pre(8) | vv_pre(8)]
                    abv = sa.tile([P, 24], F32, tag="abv", name="abv")
                    nc.vector.tensor_copy(abv[:, 0:8], pvs[:, 0:24:3])
                    nc.vector.tensor_copy(abv[:, 8:16], pvs[:, 24:48:3])
                    nc.vector.tensor_add(abv[:, 16:24], pvs[:, 1:24:3],
                                         pvs[:, 26:48:3])
                    tmp = sa.tile([P, 24], F32, tag="abt", name="abt")
                    nc.vector.tensor_scalar_mul(tmp[:], abv[:], 0.01)
                    nc.vector.tensor_max(abv[:], abv[:], tmp[:])
                    abve = sa.tile([P, 24], F32, tag="abve", name="abve")
                    nc.scalar.activation(abve[:], abv[:], Exp)
                    sl = slice(8 * g, 8 * g + 8)
                    nc.vector.tensor_copy(aepm[:, sl], abve[:, 0:8])
                    nc.vector.tensor_copy(bepm[:, sl], abve[:, 8:16])
                    nc.vector.tensor_scalar(fpm[:, sl], abve[:, 16:24],
                                            gcol[:, 1:2], gcol[:, 0:1],
                                            op0=MULT, op1=ADD)

                # ---- adj transposes -> atg ----
                atg = atgp.tile([P, NST, RPC], BF16, tag="atg", name="atg")
                for t in range(NST):
                    pst = ps_at.tile([P, RPC], BF16, tag="pst", name="pst")
                    for ib in range(MB):
                        nc.tensor.transpose(pst[:, ib * P:(ib + 1) * P],
                                            asb[:, ib, t * P:(t + 1) * P],
                                            ident[:])
                    if t % 2 == 0:
                        nc.scalar.copy(atg[:, t, :], pst[:])
                    else:
                        nc.vector.tensor_copy(atg[:, t, :], pst[:])

                # ---- outb f-scaling for this group's chunks ----
                for t in range(NST):
                    kc = NST * g + t
                    m = kc % 32
                    nc.vector.tensor_scalar_mul(outb[:, kc, :FOUT],
                                                outb[:, kc, :FOUT],
                                                fpm[:, m:m + 1])
                    nc.vector.tensor_copy(outb[:, kc, FOUT:FOUT + 1],
                                          fpm[:, m:m + 1])

                # ---- stage B matmuls ----
                h = 0 if g < 4 else 1
                for mb in range(MB):
                    yp = ps_y.tile([P, FOUT + 1], F32, tag="yp", name="yp")
                    for t in range(NST):
                        kc = NST * g + t
                        nc.tensor.matmul(yp[:],
                                         atg[:, t, mb * P:(mb + 1) * P],
                                         outb[:, kc, :],
                                         start=(t == 0), stop=(t == NST - 1))
                    if g % 4 == 0:
                        nc.vector.tensor_copy(ysb[h][:, mb, :], yp[:])
                    else:
                        nc.vector.tensor_add(ysb[h][:, mb, :],
                                             ysb[h][:, mb, :], yp[:])

            # ---- alphas ----
            for mb in range(MB):
                m1 = sa.tile([P, 32], F32, tag="alm", name="alm1")
                nc.vector.tensor_mul(m1[:], aepm[:], selg[:, mb, :])
                nc.vector.tensor_reduce(al1[:, mb:mb + 1], m1[:], axis=AX,
                                        op=ADD)
                m2 = sa.tile([P, 32], F32, tag="alm", name="alm2")
                nc.vector.tensor_mul(m2[:], bepm[:], selg[:, mb, :])
                nc.vector.tensor_reduce(al2[:, mb:mb + 1], m2[:], axis=AX,
                                        op=ADD)
            nc.vector.tensor_scalar_add(al1[:], al1[:], gcol[:, 1:2])
            nc.vector.tensor_scalar_add(al2[:], al2[:], gcol[:, 1:2])

            # ---- combine + sigmoid + store ----
            for mb in range(MB):
                z1 = sa.tile([P, FOUT + 1], F32, tag="z1", name="z1")
                z2 = sa.tile([P, FOUT + 1], F32, tag="z2", name="z2")
                nc.vector.tensor_scalar_mul(z1[:], ysb[0][:, mb, :],
                                            al1[:, mb:mb + 1])
                nc.vector.tensor_scalar_mul(z2[:], ysb[1][:, mb, :],
                                            al2[:, mb:mb + 1])
                nc.vector.tensor_add(z1[:], z1[:], z2[:])
                rec = sa.tile([P, 1], F32, tag="rec", name="rec")
                nc.vector.reciprocal(rec[:], z1[:, FOUT:FOUT + 1])
                res = sa.tile([P, FOUT], F32, tag="res", name="res")
                nc.vector.tensor_scalar_mul(res[:], z1[:, :FOUT], rec[:])
                resg = sa.tile([P, FOUT], F32, tag="resg", name="resg")
                nc.scalar.activation(resg[:], res[:], Sigmoid)
                nc.sync.dma_start(y_d.ap()[mb * P:(mb + 1) * P, :], resg[:])

    nc.compile()
    return nc


_NC_CACHE = None


def _get_program():
    global _NC_CACHE
    if _NC_CACHE is None:
        _NC_CACHE = build_program()
    return _NC_CACHE


def _to_bf16(a):
    return np.ascontiguousarray(np.asarray(a, np.float32)).astype(
        ml_dtypes.bfloat16)


def make_in_maps(x, weight, att_vec, adj):
    x = np.asarray(x, dtype=np.float32)
    weight = np.ascontiguousarray(np.asarray(weight, dtype=np.float32))
    att_vec = np.asarray(att_vec, dtype=np.float32)
    adj = np.asarray(adj, dtype=np.int32)

    xt = _to_bf16(x.T)                             # [512, 8192] bf16
    attb = np.broadcast_to(att_vec[:, 0][None, :], (P, 2 * FOUT)).copy()

    emat = np.zeros((P, 4, P), np.float32)
    for i in range(P):
        if i < 64:
            emat[2 * i, 0, i] = 1
            emat[2 * i + 1, 2, i] = 1
        else:
            emat[2 * i - 128, 1, i] = 1
            emat[2 * i - 127, 3, i] = 1
    emat = _to_bf16(emat)

    in_maps = []
    for c in range(NCORES):
        g = 1.0 if c < 4 else 0.0
        gcol = np.empty((P, 2), np.float32)
        gcol[:, 0] = g
        gcol[:, 1] = 1.0 - g
        selg = np.zeros((P, MB, 32), np.float32)
        for mb in range(MB):
            selg[:, mb, (8 * c + mb) % 32] = g
        in_maps.append({
            "xt": xt,
            "w": weight,
            "attb": attb,
            "adj": np.ascontiguousarray(adj[c * RPC:(c + 1) * RPC, :]),
            "gcol": gcol,
            "selg": selg,
            "emat": emat,
        })
    return in_maps


def kernel(x, weight, att_vec, adj, _trace=False, _trace_kwargs=None):
    nc = _get_program()
    in_maps = make_in_maps(x, weight, att_vec, adj)
    r = run_bass_kernel_spmd(nc, in_maps, core_ids=list(range(NCORES)),
                             trace=_trace, **(_trace_kwargs or {}))
    y = np.concatenate([r.results[c]["y"] for c in range(NCORES)], axis=0)
    kernel.last_results = r
    return y.astype(np.float32)


# revision 7
# speedup vs baseline: 2.2722x; 1.3336x over previous
"""GAT layer (nn_GAT_layer_67619965108552) as a Trainium2 Bass/Tile SPMD kernel.

Structure exploited (validated vs reference to 5.2e-3 in bf16):
  With n=8192, the buggy-but-faithful pair indexing collapses:
    rows i < 4096:  scores[i, j] = u[2i + (j >= 4096)],  u = x @ (W@a1 + W@a2)
    rows i >= 4096: scores[i, j] = tt[j mod 4096],  tt[q] = s1[2q] + s2[2q+1]
  After leaky_relu + adj masking + softmax, attn @ out reduces to masked
  row-sum matmuls against f-scaled out:
    ysb[h][mb] = sum_{kc in half h} adjT(mb,kc)^T @ (f * [out | 1])[kc]
    res = sigmoid((al1*ysbL + al2*ysbR)[:, :256] / (...)[:, 256])
  Top-half cores: f = 1, al1/al2 = exp(lrelu(u at even/odd rows)); bottom-half
  cores: f = exp(lrelu(tt)), al = 1. One instruction stream for all cores;
  divergence is data-driven (g flag / select masks).

Single fused pipeline in 8 super-tiles: x^T (bf16) streams on the HWDGE queue
while the per-core transposed adj slice (int32 -> bf16 cast) streams on the
SWDGE queue in 4MB column-group DMAs with 4KB-contiguous runs — no on-chip
transposes at all; u/s1/s2 come free as 3 extra matmul columns; per-pair
score vectors are extracted with constant 0/1 parity-pick matmuls; f-scaling
is per-chunk so nothing serializes globally.

Sharding: rows of adj / output across 8 cores (1024 each); x/weight/att_vec
replicated (each core computes the full out = x@W as the rhs of its matmuls).
"""
import ml_dtypes
import numpy as np
from contextlib import ExitStack

import concourse.bass as bass
import concourse.tile as tile
from concourse import bacc, mybir
from concourse.bass_utils import run_bass_kernel_spmd

F32 = mybir.dt.float32
BF16 = mybir.dt.bfloat16
I32 = mybir.dt.int32

N = 8192           # nodes
FIN = 512          # input features
FOUT = 256         # output features
P = 128
NB = N // P        # 64 row-blocks of out
NCORES = 8
RPC = N // NCORES  # 1024 rows per core
MB = RPC // P      # 8 output row-blocks per core
NST = 8            # super-tiles (adj column groups of 1024)


def build_program():
    nc = bacc.Bacc("TRN2", target_bir_lowering=False, debug=False,
                   num_devices=NCORES)

    xt_d = nc.dram_tensor("xt", [FIN, N], BF16, kind="ExternalInput")
    w_d = nc.dram_tensor("w", [FIN, FOUT], F32, kind="ExternalInput")
    attb_d = nc.dram_tensor("attb", [P, 2 * FOUT], F32, kind="ExternalInput")
    # per-core transposed adj slice: adjt[j, r] = adj[c*RPC + r, j]
    adjt_d = nc.dram_tensor("adjt", [N, RPC], I32, kind="ExternalInput")
    # gcol[:, 0] = g (1.0 top-half cores, 0.0 bottom), gcol[:, 1] = 1-g
    g_d = nc.dram_tensor("gcol", [P, 2], F32, kind="ExternalInput")
    # selg[p, mb, m] = g * (m == 8c + mb) : per-core pair select for alphas
    selg_d = nc.dram_tensor("selg", [P, MB, 32], F32, kind="ExternalInput")
    # parity-pick matrices [E0 | E1 | Eo0 | Eo1]
    emat_d = nc.dram_tensor("emat", [P, 4, P], BF16, kind="ExternalInput")
    y_d = nc.dram_tensor("y", [RPC, FOUT], F32, kind="ExternalOutput")

    Exp = mybir.ActivationFunctionType.Exp
    Sigmoid = mybir.ActivationFunctionType.Sigmoid
    AX = mybir.AxisListType.X
    ADD = mybir.AluOpType.add
    MULT = mybir.AluOpType.mult

    with tile.TileContext(nc) as tc, ExitStack() as ctx:
        constp = ctx.enter_context(tc.tile_pool(name="const", bufs=1))

        # ---- constants (order matters: wtile/attb first, xts0 early) ----
        wtile = constp.tile([P, 4, FOUT], F32)
        nc.sync.dma_start(wtile[:], w_d.ap().rearrange("(c p) f -> p c f", p=P))
        attb = constp.tile([P, 2 * FOUT], F32)
        nc.sync.dma_start(attb[:], attb_d.ap())

        # persistent tensors
        wrhs = constp.tile([P, 4, FOUT + 3], BF16)   # [W | wu | w1 | w2]
        outb = constp.tile([P, NB, FOUT + 1], BF16)  # f*[out | 1] chunks
        aepm = constp.tile([P, 32], F32)
        bepm = constp.tile([P, 32], F32)
        fpm = constp.tile([P, 32], F32)
        al1 = constp.tile([P, MB], F32)
        al2 = constp.tile([P, MB], F32)
        ysb = [constp.tile([P, MB, FOUT + 1], F32, name=f"ysb{h}")
               for h in range(2)]
        emat = constp.tile([P, 4, P], BF16)
        gcol = constp.tile([P, 2], F32)
        selg = constp.tile([P, MB, 32], F32)

        with tc.tile_pool(name="sa", bufs=4) as sa, \
             tc.tile_pool(name="xtp", bufs=2) as xtp, \
             tc.tile_pool(name="atgp", bufs=3) as atgp, \
             tc.tile_pool(name="ps_po", bufs=2, space="PSUM") as ps_po, \
             tc.tile_pool(name="ps_y", bufs=2, space="PSUM") as ps_y, \
             tc.tile_pool(name="ps_u", bufs=2, space="PSUM") as ps_u:

            # first x chunk + first adj group start moving immediately
            xts_list = []
            xts0 = xtp.tile([P, 4, 16 * P], BF16, tag="xts", name="xts")
            nc.sync.dma_start(
                xts0[:],
                xt_d.ap()[:, 0:2048].rearrange("(c p) r -> p c r", p=P))
            xts_list.append(xts0)
            # small consts after the critical x chunk
            nc.sync.dma_start(emat[:], emat_d.ap())
            nc.sync.dma_start(gcol[:], g_d.ap())
            nc.sync.dma_start(selg[:], selg_d.ap())

            # build wrhs = [W | W@(a1+a2) | W@a1 | W@a2] per k-chunk, bf16
            wamf = sa.tile([P, 4, 3], F32, tag="wamf", name="wamf")
            for c in range(4):
                t1 = sa.tile([P, FOUT], F32, tag="wa_tmp", name="wa1")
                nc.vector.tensor_mul(t1[:], wtile[:, c, :], attb[:, :FOUT])
                nc.vector.tensor_reduce(wamf[:, c, 1:2], t1[:], axis=AX, op=ADD)
                t2 = sa.tile([P, FOUT], F32, tag="wa_tmp", name="wa2")
                nc.vector.tensor_mul(t2[:], wtile[:, c, :], attb[:, FOUT:])
                nc.vector.tensor_reduce(wamf[:, c, 2:3], t2[:], axis=AX, op=ADD)
                nc.vector.tensor_add(wamf[:, c, 0:1], wamf[:, c, 1:2],
                                     wamf[:, c, 2:3])
                nc.vector.tensor_copy(wrhs[:, c, :FOUT], wtile[:, c, :])
                nc.vector.tensor_copy(wrhs[:, c, FOUT:], wamf[:, c, :])

            for g in range(NST):
                # ---- DMAs for this super-tile ----
                if 0 < g < 4:
                    xts = xtp.tile([P, 4, 16 * P], BF16, tag="xts", name="xts")
                    nc.sync.dma_start(
                        xts[:],
                        xt_d.ap()[:, g * 2048:(g + 1) * 2048].rearrange(
                            "(c p) r -> p c r", p=P))
                    xts_list.append(xts)
                # adjT group: [128 j-part, 8 chunks, 1024 rows], i32 -> bf16
                atg = atgp.tile([P, NST, RPC], BF16, tag="atg", name="atg")
                nc.gpsimd.dma_start(
                    atg[:],
                    adjt_d.ap()[g * 1024:(g + 1) * 1024, :].rearrange(
                        "(t p) r -> p t r", p=P))

                # ---- stage A: out chunks + u/s1/s2 for 16 x-blocks ----
                if g < 4:
                    xts = xts_list[g]
                    usbE = sa.tile([P, 24], BF16, tag="usbE", name="usbE")
                    usbO = sa.tile([P, 24], BF16, tag="usbO", name="usbO")
                    for pair in range(8):
                        for half in range(2):
                            lb = 2 * pair + half      # local block 0..15
                            b = 16 * g + lb           # global block
                            po = ps_po.tile([P, FOUT + 3], F32, tag="po",
                                            name="po")
                            for c in range(4):
                                nc.tensor.matmul(
                                    po[:], xts[:, c, lb * P:(lb + 1) * P],
                                    wrhs[:, c, :],
                                    start=(c == 0), stop=(c == 3))
                            if b % 2 == 0:
                                nc.vector.tensor_copy(outb[:, b, :FOUT],
                                                      po[:, :FOUT])
                            else:
                                nc.scalar.copy(outb[:, b, :FOUT],
                                               po[:, :FOUT])
                            dst = usbE if half == 0 else usbO
                            nc.vector.tensor_copy(
                                dst[:, pair * 3:(pair + 1) * 3],
                                po[:, FOUT:FOUT + 3])
                    # parity-pick matmuls: even picks cols 0:24, odd 24:48
                    pv = ps_u.tile([P, 48], F32, tag="pv", name="pv")
                    nc.tensor.matmul(pv[:, 0:24], emat[:, 0, :], usbE[:],
                                     start=True, stop=False)
                    nc.tensor.matmul(pv[:, 0:24], emat[:, 1, :], usbO[:],
                                     start=False, stop=True)
                    nc.tensor.matmul(pv[:, 24:48], emat[:, 2, :], usbE[:],
                                     start=True, stop=False)
                    nc.tensor.matmul(pv[:, 24:48], emat[:, 3, :], usbO[:],
                                     start=False, stop=True)
                    pvs = sa.tile([P, 48], F32, tag="pvs", name="pvs")
                    nc.vector.tensor_copy(pvs[:], pv[:])
                    # abv = [ae_pre(8) | be_pre(8) | vv_pre(8)]
                    abv = sa.tile([P, 24], F32, tag="abv", name="abv")
                    nc.vector.tensor_copy(abv[:, 0:8], pvs[:, 0:24:3])
                    nc.vector.tensor_copy(abv[:, 8:16], pvs[:, 24:48:3])
                    nc.vector.tensor_add(abv[:, 16:24], pvs[:, 1:24:3],
                                         pvs[:, 26:48:3])
                    tmp = sa.tile([P, 24], F32, tag="abt", name="abt")
                    nc.vector.tensor_scalar_mul(tmp[:], abv[:], 0.01)
                    nc.vector.tensor_max(abv[:], abv[:], tmp[:])
                    abve = sa.tile([P, 24], F32, tag="abve", name="abve")
                    nc.scalar.activation(abve[:], abv[:], Exp)
                    sl = slice(8 * g, 8 * g + 8)
                    nc.vector.tensor_copy(aepm[:, sl], abve[:, 0:8])
                    nc.vector.tensor_copy(bepm[:, sl], abve[:, 8:16])
                    nc.vector.tensor_scalar(fpm[:, sl], abve[:, 16:24],
                                            gcol[:, 1:2], gcol[:, 0:1],
                                            op0=MULT, op1=ADD)

                # ---- outb f-scaling for this group's chunks ----
                klo = NST * g
                m0 = klo % 32
                for t in range(NST):
                    kc = klo + t
                    m = kc % 32
                    nc.vector.tensor_scalar_mul(outb[:, kc, :FOUT],
                                                outb[:, kc, :FOUT],
                                                fpm[:, m:m + 1])
                nc.vector.tensor_copy(outb[:, klo:klo + NST, FOUT:FOUT + 1],
                                      fpm[:, m0:m0 + NST])

                # ---- stage B matmuls ----
                h = 0 if g < 4 else 1
                for mb in range(MB):
                    yp = ps_y.tile([P, FOUT + 1], F32, tag="yp", name="yp")
                    for t in range(NST):
                        kc = klo + t
                        nc.tensor.matmul(yp[:],
                                         atg[:, t, mb * P:(mb + 1) * P],
                                         outb[:, kc, :],
                                         start=(t == 0), stop=(t == NST - 1))
                    if g % 4 == 0:
                        nc.vector.tensor_copy(ysb[h][:, mb, :], yp[:])
                    else:
                        nc.vector.tensor_add(ysb[h][:, mb, :],
                                             ysb[h][:, mb, :], yp[:])

            # ---- alphas ----
            for mb in range(MB):
                m1 = sa.tile([P, 32], F32, tag="alm", name="alm1")
                nc.vector.tensor_mul(m1[:], aepm[:], selg[:, mb, :])
                nc.vector.tensor_reduce(al1[:, mb:mb + 1], m1[:], axis=AX,
                                        op=ADD)
                m2 = sa.tile([P, 32], F32, tag="alm", name="alm2")
                nc.vector.tensor_mul(m2[:], bepm[:], selg[:, mb, :])
                nc.vector.tensor_reduce(al2[:, mb:mb + 1], m2[:], axis=AX,
                                        op=ADD)
            nc.vector.tensor_scalar_add(al1[:], al1[:], gcol[:, 1:2])
            nc.vector.tensor_scalar_add(al2[:], al2[:], gcol[:, 1:2])

            # ---- combine + sigmoid + store ----
            for mb in range(MB):
                z1 = sa.tile([P, FOUT + 1], F32, tag="z1", name="z1")
                z2 = sa.tile([P, FOUT + 1], F32, tag="z2", name="z2")
                nc.vector.tensor_scalar_mul(z1[:], ysb[0][:, mb, :],
                                            al1[:, mb:mb + 1])
                nc.vector.tensor_scalar_mul(z2[:], ysb[1][:, mb, :],
                                            al2[:, mb:mb + 1])
                nc.vector.tensor_add(z1[:], z1[:], z2[:])
                rec = sa.tile([P, 1], F32, tag="rec", name="rec")
                nc.vector.reciprocal(rec[:], z1[:, FOUT:FOUT + 1])
                res = sa.tile([P, FOUT], F32, tag="res", name="res")
                nc.vector.tensor_scalar_mul(res[:], z1[:, :FOUT], rec[:])
                resg = sa.tile([P, FOUT], F32, tag="resg", name="resg")
                nc.scalar.activation(resg[:], res[:], Sigmoid)
                nc.sync.dma_start(y_d.ap()[mb * P:(mb + 1) * P, :], resg[:])

    nc.compile()
    return nc


_NC_CACHE = None


def _get_program():
    global _NC_CACHE
    if _NC_CACHE is None:
        _NC_CACHE = build_program()
    return _NC_CACHE


def _to_bf16(a):
    return np.ascontiguousarray(np.asarray(a, np.float32)).astype(
        ml_dtypes.bfloat16)


def make_in_maps(x, weight, att_vec, adj):
    x = np.asarray(x, dtype=np.float32)
    weight = np.ascontiguousarray(np.asarray(weight, dtype=np.float32))
    att_vec = np.asarray(att_vec, dtype=np.float32)
    adj = np.asarray(adj, dtype=np.int32)

    xt = _to_bf16(x.T)                             # [512, 8192] bf16
    attb = np.broadcast_to(att_vec[:, 0][None, :], (P, 2 * FOUT)).copy()
    adjT = np.ascontiguousarray(adj.T)             # [8192, 8192] int32

    emat = np.zeros((P, 4, P), np.float32)
    for i in range(P):
        if i < 64:
            emat[2 * i, 0, i] = 1
            emat[2 * i + 1, 2, i] = 1
        else:
            emat[2 * i - 128, 1, i] = 1
            emat[2 * i - 127, 3, i] = 1
    emat = _to_bf16(emat)

    in_maps = []
    for c in range(NCORES):
        g = 1.0 if c < 4 else 0.0
        gcol = np.empty((P, 2), np.float32)
        gcol[:, 0] = g
        gcol[:, 1] = 1.0 - g
        selg = np.zeros((P, MB, 32), np.float32)
        for mb in range(MB):
            selg[:, mb, (8 * c + mb) % 32] = g
        in_maps.append({
            "xt": xt,
            "w": weight,
            "attb": attb,
            "adjt": np.ascontiguousarray(adjT[:, c * RPC:(c + 1) * RPC]),
            "gcol": gcol,
            "selg": selg,
            "emat": emat,
        })
    return in_maps


def kernel(x, weight, att_vec, adj, _trace=False, _trace_kwargs=None):
    nc = _get_program()
    in_maps = make_in_maps(x, weight, att_vec, adj)
    r = run_bass_kernel_spmd(nc, in_maps, core_ids=list(range(NCORES)),
                             trace=_trace, **(_trace_kwargs or {}))
    y = np.concatenate([r.results[c]["y"] for c in range(NCORES)], axis=0)
    kernel.last_results = r
    return y.astype(np.float32)


# revision 10
# speedup vs baseline: 2.3268x; 1.0240x over previous
"""GAT layer (nn_GAT_layer_67619965108552) as a Trainium2 Bass/Tile SPMD kernel.

Structure exploited (validated vs reference to 5.2e-3 in bf16):
  With n=8192, the buggy-but-faithful pair indexing collapses:
    rows i < 4096:  scores[i, j] = u[2i + (j >= 4096)],  u = x @ (W@a1 + W@a2)
    rows i >= 4096: scores[i, j] = tt[j mod 4096],  tt[q] = s1[2q] + s2[2q+1]
  After leaky_relu + adj masking + softmax, attn @ out reduces to masked
  row-sum matmuls against f-scaled out:
    ysb[h][mb] = sum_{kc in half h} adjT(mb,kc)^T @ (f * [out | 1])[kc]
    res = sigmoid((al1*ysbL + al2*ysbR)[:, :256] / (...)[:, 256])
  Top-half cores: f = 1, al1/al2 = exp(lrelu(u at even/odd rows)); bottom-half
  cores: f = exp(lrelu(tt)), al = 1. One instruction stream for all cores;
  divergence is data-driven (g flag / select masks).

Single fused pipeline in 8 super-tiles: x^T (bf16) streams on the HWDGE queue
while the per-core transposed adj slice (int32 -> bf16 cast) streams on the
SWDGE queue in 4MB column-group DMAs with 4KB-contiguous runs — no on-chip
transposes at all; u/s1/s2 come free as 3 extra matmul columns; per-pair
score vectors are extracted with constant 0/1 parity-pick matmuls; f-scaling
is per-chunk so nothing serializes globally.

Sharding: rows of adj / output across 8 cores (1024 each); x/weight/att_vec
replicated (each core computes the full out = x@W as the rhs of its matmuls).
"""
import ml_dtypes
import numpy as np
from contextlib import ExitStack

import concourse.bass as bass
import concourse.tile as tile
from concourse import bacc, mybir
from concourse.bass_utils import run_bass_kernel_spmd

F32 = mybir.dt.float32
BF16 = mybir.dt.bfloat16
I32 = mybir.dt.int32

N = 8192           # nodes
FIN = 512          # input features
FOUT = 256         # output features
P = 128
NB = N // P        # 64 row-blocks of out
NCORES = 8
RPC = N // NCORES  # 1024 rows per core
MB = RPC // P      # 8 output row-blocks per core
NST = 8            # super-tiles (adj column groups of 1024)


def build_program():
    nc = bacc.Bacc("TRN2", target_bir_lowering=False, debug=False,
                   num_devices=NCORES)

    xt_d = nc.dram_tensor("xt", [FIN, N], BF16, kind="ExternalInput")
    w_d = nc.dram_tensor("w", [FIN, FOUT], F32, kind="ExternalInput")
    attb_d = nc.dram_tensor("attb", [P, 2 * FOUT], F32, kind="ExternalInput")
    # per-core transposed adj slice: adjt[j, r] = adj[c*RPC + r, j]
    adjt_d = nc.dram_tensor("adjt", [N, RPC], I32, kind="ExternalInput")
    # gcol[:, 0] = g (1.0 top-half cores, 0.0 bottom), gcol[:, 1] = 1-g
    g_d = nc.dram_tensor("gcol", [P, 2], F32, kind="ExternalInput")
    # selg[p, mb, m] = g * (m == 8c + mb) : per-core pair select for alphas
    selg_d = nc.dram_tensor("selg", [P, MB, 32], F32, kind="ExternalInput")
    # parity-pick matrices [E0 | E1 | Eo0 | Eo1]
    emat_d = nc.dram_tensor("emat", [P, 4, P], BF16, kind="ExternalInput")
    y_d = nc.dram_tensor("y", [RPC, FOUT], F32, kind="ExternalOutput")

    Exp = mybir.ActivationFunctionType.Exp
    Sigmoid = mybir.ActivationFunctionType.Sigmoid
    AX = mybir.AxisListType.X
    ADD = mybir.AluOpType.add
    MULT = mybir.AluOpType.mult

    with tile.TileContext(nc) as tc, ExitStack() as ctx:
        constp = ctx.enter_context(tc.tile_pool(name="const", bufs=1))

        # ---- constants (order matters: wtile/attb first, xts0 early) ----
        wtile = constp.tile([P, 4, FOUT], F32)
        nc.sync.dma_start(wtile[:], w_d.ap().rearrange("(c p) f -> p c f", p=P))
        attb = constp.tile([P, 2 * FOUT], F32)
        nc.sync.dma_start(attb[:], attb_d.ap())

        # persistent tensors
        wrhs = constp.tile([P, 4, FOUT + 3], BF16)   # [W | wu | w1 | w2]
        outb = constp.tile([P, NB, FOUT + 1], BF16)  # f*[out | 1] chunks
        aepm = constp.tile([P, 32], F32)
        bepm = constp.tile([P, 32], F32)
        fpm = constp.tile([P, 32], F32)
        al1 = constp.tile([P, MB], F32)
        al2 = constp.tile([P, MB], F32)
        ysb = [constp.tile([P, MB, FOUT + 1], F32, name=f"ysb{h}")
               for h in range(2)]
        emat = constp.tile([P, 4, P], BF16)
        gcol = constp.tile([P, 2], F32)
        selg = constp.tile([P, MB, 32], F32)

        with tc.tile_pool(name="sa", bufs=4) as sa, \
             tc.tile_pool(name="xtp", bufs=3) as xtp, \
             tc.tile_pool(name="atgp", bufs=4) as atgp, \
             tc.tile_pool(name="ps_po", bufs=2, space="PSUM") as ps_po, \
             tc.tile_pool(name="ps_y", bufs=2, space="PSUM") as ps_y, \
             tc.tile_pool(name="ps_u", bufs=2, space="PSUM") as ps_u:

            # first x chunk (in two halves for earlier first block) starts
            # moving immediately
            xts_list = []
            xts0 = xtp.tile([P, 4, 16 * P], BF16, tag="xts", name="xts")
            for hh in range(2):
                nc.sync.dma_start(
                    xts0[:, :, hh * 1024:(hh + 1) * 1024],
                    xt_d.ap()[:, hh * 1024:(hh + 1) * 1024].rearrange(
                        "(c p) r -> p c r", p=P))
            xts_list.append(xts0)
            # small consts after the critical x chunk
            nc.sync.dma_start(emat[:], emat_d.ap())
            nc.sync.dma_start(gcol[:], g_d.ap())
            nc.sync.dma_start(selg[:], selg_d.ap())

            # build wrhs = [W | W@(a1+a2) | W@a1 | W@a2] per k-chunk, bf16
            wamf = sa.tile([P, 4, 3], F32, tag="wamf", name="wamf")
            for c in range(4):
                t1 = sa.tile([P, FOUT], F32, tag="wa_tmp", name="wa1")
                nc.vector.tensor_mul(t1[:], wtile[:, c, :], attb[:, :FOUT])
                nc.vector.tensor_reduce(wamf[:, c, 1:2], t1[:], axis=AX, op=ADD)
                t2 = sa.tile([P, FOUT], F32, tag="wa_tmp", name="wa2")
                nc.vector.tensor_mul(t2[:], wtile[:, c, :], attb[:, FOUT:])
                nc.vector.tensor_reduce(wamf[:, c, 2:3], t2[:], axis=AX, op=ADD)
                nc.vector.tensor_add(wamf[:, c, 0:1], wamf[:, c, 1:2],
                                     wamf[:, c, 2:3])
                nc.vector.tensor_copy(wrhs[:, c, :FOUT], wtile[:, c, :])
                nc.vector.tensor_copy(wrhs[:, c, FOUT:], wamf[:, c, :])

            atg_list = []

            def stage_a(g):
                xts = xts_list[g]
                usbE = sa.tile([P, 24], BF16, tag="usbE", name="usbE")
                usbO = sa.tile([P, 24], BF16, tag="usbO", name="usbO")
                for pair in range(8):
                    for half in range(2):
                        lb = 2 * pair + half      # local block 0..15
                        b = 16 * g + lb           # global block
                        po = ps_po.tile([P, FOUT + 3], F32, tag="po",
                                        name="po")
                        for c in range(4):
                            nc.tensor.matmul(
                                po[:], xts[:, c, lb * P:(lb + 1) * P],
                                wrhs[:, c, :],
                                start=(c == 0), stop=(c == 3))
                        nc.vector.tensor_copy(outb[:, b, :FOUT],
                                              po[:, :FOUT])
                        dst = usbE if half == 0 else usbO
                        nc.scalar.copy(dst[:, pair * 3:(pair + 1) * 3],
                                       po[:, FOUT:FOUT + 3])
                # parity-pick matmuls: even picks cols 0:24, odd 24:48
                pv = ps_u.tile([P, 48], F32, tag="pv", name="pv")
                nc.tensor.matmul(pv[:, 0:24], emat[:, 0, :], usbE[:],
                                 start=True, stop=False)
                nc.tensor.matmul(pv[:, 0:24], emat[:, 1, :], usbO[:],
                                 start=False, stop=True)
                nc.tensor.matmul(pv[:, 24:48], emat[:, 2, :], usbE[:],
                                 start=True, stop=False)
                nc.tensor.matmul(pv[:, 24:48], emat[:, 3, :], usbO[:],
                                 start=False, stop=True)
                pvs = sa.tile([P, 48], F32, tag="pvs", name="pvs")
                nc.vector.tensor_copy(pvs[:], pv[:])
                # abv = [ae_pre(8) | be_pre(8) | vv_pre(8)]
                abv = sa.tile([P, 24], F32, tag="abv", name="abv")
                nc.vector.tensor_copy(abv[:, 0:8], pvs[:, 0:24:3])
                nc.vector.tensor_copy(abv[:, 8:16], pvs[:, 24:48:3])
                nc.vector.tensor_add(abv[:, 16:24], pvs[:, 1:24:3],
                                     pvs[:, 26:48:3])
                tmp = sa.tile([P, 24], F32, tag="abt", name="abt")
                nc.vector.tensor_scalar_mul(tmp[:], abv[:], 0.01)
                nc.vector.tensor_max(abv[:], abv[:], tmp[:])
                abve = sa.tile([P, 24], F32, tag="abve", name="abve")
                nc.scalar.activation(abve[:], abv[:], Exp)
                sl = slice(8 * g, 8 * g + 8)
                nc.vector.tensor_copy(aepm[:, sl], abve[:, 0:8])
                nc.vector.tensor_copy(bepm[:, sl], abve[:, 8:16])
                nc.vector.tensor_scalar(fpm[:, sl], abve[:, 16:24],
                                        gcol[:, 1:2], gcol[:, 0:1],
                                        op0=MULT, op1=ADD)

            def scale_group(g):
                klo = NST * g
                m0 = klo % 32
                for t in range(NST):
                    kc = klo + t
                    m = kc % 32
                    nc.scalar.activation(outb[:, kc, :FOUT],
                                         outb[:, kc, :FOUT],
                                         mybir.ActivationFunctionType.Copy,
                                         scale=fpm[:, m:m + 1])
                nc.scalar.copy(outb[:, klo:klo + NST, FOUT:FOUT + 1],
                               fpm[:, m0:m0 + NST])

            def stage_b(g):
                klo = NST * g
                atg = atg_list[g]
                h = 0 if g < 4 else 1
                for mb in range(MB):
                    yp = ps_y.tile([P, FOUT + 1], F32, tag="yp", name="yp")
                    for t in range(NST):
                        kc = klo + t
                        nc.tensor.matmul(yp[:],
                                         atg[:, t, mb * P:(mb + 1) * P],
                                         outb[:, kc, :],
                                         start=(t == 0), stop=(t == NST - 1))
                    if g % 4 == 0:
                        nc.vector.tensor_copy(ysb[h][:, mb, :], yp[:])
                    else:
                        nc.vector.tensor_add(ysb[h][:, mb, :],
                                             ysb[h][:, mb, :], yp[:])

            # software-pipelined: stage A of super-tile s runs ahead of
            # stage B of super-tile s-1 so the PE never waits on adj DMA
            for s in range(NST):
                if 0 < s < 4:
                    xts = xtp.tile([P, 4, 16 * P], BF16, tag="xts",
                                   name="xts")
                    nc.sync.dma_start(
                        xts[:],
                        xt_d.ap()[:, s * 2048:(s + 1) * 2048].rearrange(
                            "(c p) r -> p c r", p=P))
                    xts_list.append(xts)
                # adjT group: [128 j-part, 8 chunks, 1024 rows], i32 -> bf16
                atg = atgp.tile([P, NST, RPC], BF16, tag="atg", name="atg")
                nc.gpsimd.dma_start(
                    atg[:],
                    adjt_d.ap()[s * 1024:(s + 1) * 1024, :].rearrange(
                        "(t p) r -> p t r", p=P))
                atg_list.append(atg)

                if s < 4:
                    stage_a(s)
                scale_group(s)
                if s >= 1:
                    stage_b(s - 1)
            stage_b(NST - 1)

            # ---- alphas ----
            for mb in range(MB):
                m1 = sa.tile([P, 32], F32, tag="alm", name="alm1")
                nc.vector.tensor_mul(m1[:], aepm[:], selg[:, mb, :])
                nc.vector.tensor_reduce(al1[:, mb:mb + 1], m1[:], axis=AX,
                                        op=ADD)
                m2 = sa.tile([P, 32], F32, tag="alm", name="alm2")
                nc.vector.tensor_mul(m2[:], bepm[:], selg[:, mb, :])
                nc.vector.tensor_reduce(al2[:, mb:mb + 1], m2[:], axis=AX,
                                        op=ADD)
            nc.vector.tensor_scalar_add(al1[:], al1[:], gcol[:, 1:2])
            nc.vector.tensor_scalar_add(al2[:], al2[:], gcol[:, 1:2])

            # ---- combine + sigmoid + store ----
            for mb in range(MB):
                z1 = sa.tile([P, FOUT + 1], F32, tag="z1", name="z1")
                z2 = sa.tile([P, FOUT + 1], F32, tag="z2", name="z2")
                nc.vector.tensor_scalar_mul(z1[:], ysb[0][:, mb, :],
                                            al1[:, mb:mb + 1])
                nc.vector.tensor_scalar_mul(z2[:], ysb[1][:, mb, :],
                                            al2[:, mb:mb + 1])
                nc.vector.tensor_add(z1[:], z1[:], z2[:])
                rec = sa.tile([P, 1], F32, tag="rec", name="rec")
                nc.vector.reciprocal(rec[:], z1[:, FOUT:FOUT + 1])
                res = sa.tile([P, FOUT], F32, tag="res", name="res")
                nc.vector.tensor_scalar_mul(res[:], z1[:, :FOUT], rec[:])
                resg = sa.tile([P, FOUT], F32, tag="resg", name="resg")
                nc.scalar.activation(resg[:], res[:], Sigmoid)
                nc.sync.dma_start(y_d.ap()[mb * P:(mb + 1) * P, :], resg[:])

    nc.compile()
    return nc


_NC_CACHE = None


def _get_program():
    global _NC_CACHE
    if _NC_CACHE is None:
        _NC_CACHE = build_program()
    return _NC_CACHE


def _to_bf16(a):
    return np.ascontiguousarray(np.asarray(a, np.float32)).astype(
        ml_dtypes.bfloat16)


def make_in_maps(x, weight, att_vec, adj):
    x = np.asarray(x, dtype=np.float32)
    weight = np.ascontiguousarray(np.asarray(weight, dtype=np.float32))
    att_vec = np.asarray(att_vec, dtype=np.float32)
    adj = np.asarray(adj, dtype=np.int32)

    xt = _to_bf16(x.T)                             # [512, 8192] bf16
    attb = np.broadcast_to(att_vec[:, 0][None, :], (P, 2 * FOUT)).copy()
    adjT = np.ascontiguousarray(adj.T)             # [8192, 8192] int32

    emat = np.zeros((P, 4, P), np.float32)
    for i in range(P):
        if i < 64:
            emat[2 * i, 0, i] = 1
            emat[2 * i + 1, 2, i] = 1
        else:
            emat[2 * i - 128, 1, i] = 1
            emat[2 * i - 127, 3, i] = 1
    emat = _to_bf16(emat)

    in_maps = []
    for c in range(NCORES):
        g = 1.0 if c < 4 else 0.0
        gcol = np.empty((P, 2), np.float32)
        gcol[:, 0] = g
        gcol[:, 1] = 1.0 - g
        selg = np.zeros((P, MB, 32), np.float32)
        for mb in range(MB):
            selg[:, mb, (8 * c + mb) % 32] = g
        in_maps.append({
            "xt": xt,
            "w": weight,
            "attb": attb,
            "adjt": np.ascontiguousarray(adjT[:, c * RPC:(c + 1) * RPC]),
            "gcol": gcol,
            "selg": selg,
            "emat": emat,
        })
    return in_maps


def kernel(x, weight, att_vec, adj, _trace=False, _trace_kwargs=None):
    nc = _get_program()
    in_maps = make_in_maps(x, weight, att_vec, adj)
    r = run_bass_kernel_spmd(nc, in_maps, core_ids=list(range(NCORES)),
                             trace=_trace, **(_trace_kwargs or {}))
    y = np.concatenate([r.results[c]["y"] for c in range(NCORES)], axis=0)
    kernel.last_results = r
    return y.astype(np.float32)


# revision 13
# speedup vs baseline: 2.5458x; 1.0941x over previous
"""GAT layer (nn_GAT_layer_67619965108552) as a Trainium2 Bass/Tile SPMD kernel.

Structure exploited (validated vs reference to 5.2e-3 in bf16):
  With n=8192, the buggy-but-faithful pair indexing collapses:
    rows i < 4096:  scores[i, j] = u[2i + (j >= 4096)],  u = x @ (W@a1 + W@a2)
    rows i >= 4096: scores[i, j] = tt[j mod 4096],  tt[q] = s1[2q] + s2[2q+1]
  After leaky_relu + adj masking + softmax, attn @ out reduces to masked
  row-sum matmuls against f-scaled out:
    ysb[h][mb] = sum_{kc in half h} adjT(mb,kc)^T @ (f * [out | 1])[kc]
    res = sigmoid((al1*ysbL + al2*ysbR)[:, :256] / (...)[:, 256])
  Top-half cores: f = 1, al1/al2 = exp(lrelu(u at even/odd rows)); bottom-half
  cores: f = exp(lrelu(tt)), al = 1. One instruction stream for all cores;
  divergence is data-driven (g flag / select masks).

Single fused pipeline in 8 super-tiles: x^T (bf16) streams on the HWDGE queue
while the per-core transposed adj slice (int32 -> bf16 cast) streams on the
SWDGE queue in 4MB column-group DMAs with 4KB-contiguous runs — no on-chip
transposes at all; u/s1/s2 come free as 3 extra matmul columns; per-pair
score vectors are extracted with constant 0/1 parity-pick matmuls; f-scaling
is per-chunk so nothing serializes globally.

Sharding: rows of adj / output across 8 cores (1024 each); x/weight/att_vec
replicated (each core computes the full out = x@W as the rhs of its matmuls).
"""
import ml_dtypes
import numpy as np
from contextlib import ExitStack

import concourse.bass as bass
import concourse.tile as tile
from concourse import bacc, mybir
from concourse.bass_utils import run_bass_kernel_spmd

F32 = mybir.dt.float32
BF16 = mybir.dt.bfloat16
I32 = mybir.dt.int32

N = 8192           # nodes
FIN = 512          # input features
FOUT = 256         # output features
P = 128
NB = N // P        # 64 row-blocks of out
NCORES = 8
RPC = N // NCORES  # 1024 rows per core
MB = RPC // P      # 8 output row-blocks per core
NST = 8            # super-tiles (adj column groups of 1024)


def build_program():
    nc = bacc.Bacc("TRN2", target_bir_lowering=False, debug=False,
                   num_devices=NCORES)

    xt_d = nc.dram_tensor("xt", [FIN, N], BF16, kind="ExternalInput")
    w_d = nc.dram_tensor("w", [FIN, FOUT], F32, kind="ExternalInput")
    attb_d = nc.dram_tensor("attb", [P, 2 * FOUT], F32, kind="ExternalInput")
    # per-core transposed adj slice: adjt[j, r] = adj[c*RPC + r, j]
    adjt_d = nc.dram_tensor("adjt", [N, RPC], I32, kind="ExternalInput")
    # gcol[:, 0] = g (1.0 top-half cores, 0.0 bottom), gcol[:, 1] = 1-g
    g_d = nc.dram_tensor("gcol", [P, 2], F32, kind="ExternalInput")
    # selg[p, mb, m] = g * (m == 8c + mb) : per-core pair select for alphas
    selg_d = nc.dram_tensor("selg", [P, MB, 32], F32, kind="ExternalInput")
    # parity-pick matrices [E0 | E1 | Eo0 | Eo1]
    emat_d = nc.dram_tensor("emat", [P, 4, P], BF16, kind="ExternalInput")
    y_d = nc.dram_tensor("y", [RPC, FOUT], F32, kind="ExternalOutput")

    Exp = mybir.ActivationFunctionType.Exp
    Sigmoid = mybir.ActivationFunctionType.Sigmoid
    AX = mybir.AxisListType.X
    ADD = mybir.AluOpType.add
    MULT = mybir.AluOpType.mult

    with tile.TileContext(nc) as tc, ExitStack() as ctx:
        constp = ctx.enter_context(tc.tile_pool(name="const", bufs=1))

        wtile = constp.tile([P, 4, FOUT], F32)
        attb = constp.tile([P, 2 * FOUT], F32)

        # persistent tensors
        wrhs = constp.tile([P, 4, FOUT + 3], BF16)   # [W | wu | w1 | w2]
        outb = constp.tile([P, NB, FOUT + 1], BF16)  # f*[out | 1] chunks
        aepm = constp.tile([P, 32], F32)
        bepm = constp.tile([P, 32], F32)
        fpm = constp.tile([P, 32], F32)
        al1 = constp.tile([P, MB], F32)
        al2 = constp.tile([P, MB], F32)
        ysb = [constp.tile([P, MB, FOUT + 1], F32, name=f"ysb{h}")
               for h in range(2)]
        emat = constp.tile([P, 4, P], BF16)
        gcol = constp.tile([P, 2], F32)
        selg = constp.tile([P, MB, 32], F32)

        with tc.tile_pool(name="sa", bufs=4) as sa, \
             tc.tile_pool(name="xtp", bufs=3) as xtp, \
             tc.tile_pool(name="atgp", bufs=4) as atgp, \
             tc.tile_pool(name="ps_po", bufs=2, space="PSUM") as ps_po, \
             tc.tile_pool(name="ps_y", bufs=2, space="PSUM") as ps_y, \
             tc.tile_pool(name="ps_u", bufs=2, space="PSUM") as ps_u:

            # first x half-chunk leads the HW queue, then the small consts
            # needed for wrhs, then the rest
            xts_list = []
            xts0 = xtp.tile([P, 4, 16 * P], BF16, tag="xts", name="xts")
            nc.sync.dma_start(
                xts0[:, :, 0:1024],
                xt_d.ap()[:, 0:1024].rearrange("(c p) r -> p c r", p=P))
            nc.sync.dma_start(wtile[:],
                              w_d.ap().rearrange("(c p) f -> p c f", p=P))
            nc.sync.dma_start(attb[:], attb_d.ap())
            nc.sync.dma_start(
                xts0[:, :, 1024:2048],
                xt_d.ap()[:, 1024:2048].rearrange("(c p) r -> p c r", p=P))
            xts_list.append(xts0)
            nc.sync.dma_start(emat[:], emat_d.ap())
            nc.sync.dma_start(gcol[:], g_d.ap())
            nc.sync.dma_start(selg[:], selg_d.ap())

            # build wrhs = [W | W@(a1+a2) | W@a1 | W@a2] per k-chunk, bf16
            wamf = sa.tile([P, 4, 3], F32, tag="wamf", name="wamf")
            for c in range(4):
                t1 = sa.tile([P, FOUT], F32, tag="wa_tmp", name="wa1")
                nc.vector.tensor_mul(t1[:], wtile[:, c, :], attb[:, :FOUT])
                nc.vector.tensor_reduce(wamf[:, c, 1:2], t1[:], axis=AX, op=ADD)
                t2 = sa.tile([P, FOUT], F32, tag="wa_tmp", name="wa2")
                nc.vector.tensor_mul(t2[:], wtile[:, c, :], attb[:, FOUT:])
                nc.vector.tensor_reduce(wamf[:, c, 2:3], t2[:], axis=AX, op=ADD)
                nc.vector.tensor_add(wamf[:, c, 0:1], wamf[:, c, 1:2],
                                     wamf[:, c, 2:3])
                nc.vector.tensor_copy(wrhs[:, c, :FOUT], wtile[:, c, :])
                nc.vector.tensor_copy(wrhs[:, c, FOUT:], wamf[:, c, :])

            atg_list = []

            def stage_a(g):
                xts = xts_list[g]
                usbE = sa.tile([P, 24], BF16, tag="usbE", name="usbE")
                usbO = sa.tile([P, 24], BF16, tag="usbO", name="usbO")
                for pair in range(8):
                    for half in range(2):
                        lb = 2 * pair + half      # local block 0..15
                        b = 16 * g + lb           # global block
                        po = ps_po.tile([P, FOUT + 3], F32, tag="po",
                                        name="po")
                        for c in range(4):
                            nc.tensor.matmul(
                                po[:], xts[:, c, lb * P:(lb + 1) * P],
                                wrhs[:, c, :],
                                start=(c == 0), stop=(c == 3))
                        nc.vector.tensor_copy(outb[:, b, :FOUT],
                                              po[:, :FOUT])
                        dst = usbE if half == 0 else usbO
                        nc.scalar.copy(dst[:, pair * 3:(pair + 1) * 3],
                                       po[:, FOUT:FOUT + 3])
                # parity-pick matmuls: even picks cols 0:24, odd 24:48
                pv = ps_u.tile([P, 48], F32, tag="pv", name="pv")
                nc.tensor.matmul(pv[:, 0:24], emat[:, 0, :], usbE[:],
                                 start=True, stop=False)
                nc.tensor.matmul(pv[:, 0:24], emat[:, 1, :], usbO[:],
                                 start=False, stop=True)
                nc.tensor.matmul(pv[:, 24:48], emat[:, 2, :], usbE[:],
                                 start=True, stop=False)
                nc.tensor.matmul(pv[:, 24:48], emat[:, 3, :], usbO[:],
                                 start=False, stop=True)
                pvs = sa.tile([P, 48], F32, tag="pvs", name="pvs")
                nc.vector.tensor_copy(pvs[:], pv[:])
                # abv = [ae_pre(8) | be_pre(8) | vv_pre(8)]
                abv = sa.tile([P, 24], F32, tag="abv", name="abv")
                nc.vector.tensor_copy(abv[:, 0:8], pvs[:, 0:24:3])
                nc.vector.tensor_copy(abv[:, 8:16], pvs[:, 24:48:3])
                nc.vector.tensor_add(abv[:, 16:24], pvs[:, 1:24:3],
                                     pvs[:, 26:48:3])
                tmp = sa.tile([P, 24], F32, tag="abt", name="abt")
                nc.vector.tensor_scalar_mul(tmp[:], abv[:], 0.01)
                nc.vector.tensor_max(abv[:], abv[:], tmp[:])
                abve = sa.tile([P, 24], F32, tag="abve", name="abve")
                nc.scalar.activation(abve[:], abv[:], Exp)
                sl = slice(8 * g, 8 * g + 8)
                nc.vector.tensor_copy(aepm[:, sl], abve[:, 0:8])
                nc.vector.tensor_copy(bepm[:, sl], abve[:, 8:16])
                nc.vector.tensor_scalar(fpm[:, sl], abve[:, 16:24],
                                        gcol[:, 1:2], gcol[:, 0:1],
                                        op0=MULT, op1=ADD)

            def scale_group(g):
                klo = NST * g
                m0 = klo % 32
                for t in range(NST):
                    kc = klo + t
                    m = kc % 32
                    nc.scalar.activation(outb[:, kc, :FOUT],
                                         outb[:, kc, :FOUT],
                                         mybir.ActivationFunctionType.Copy,
                                         scale=fpm[:, m:m + 1])
                nc.scalar.copy(outb[:, klo:klo + NST, FOUT:FOUT + 1],
                               fpm[:, m0:m0 + NST])

            def stage_b(g):
                klo = NST * g
                atg = atg_list[g]
                h = 0 if g < 4 else 1
                for mb in range(MB):
                    yp = ps_y.tile([P, FOUT + 1], F32, tag="yp", name="yp")
                    for t in range(NST):
                        kc = klo + t
                        nc.tensor.matmul(yp[:],
                                         atg[:, t, mb * P:(mb + 1) * P],
                                         outb[:, kc, :],
                                         start=(t == 0), stop=(t == NST - 1))
                    if g % 4 == 0:
                        nc.vector.tensor_copy(ysb[h][:, mb, :], yp[:])
                    else:
                        nc.vector.tensor_add(ysb[h][:, mb, :],
                                             ysb[h][:, mb, :], yp[:])

            # software-pipelined: stage A of super-tile s runs ahead of
            # stage B of super-tile s-1 so the PE never waits on adj DMA
            for s in range(NST):
                if 0 < s < 4:
                    xts = xtp.tile([P, 4, 16 * P], BF16, tag="xts",
                                   name="xts")
                    nc.sync.dma_start(
                        xts[:],
                        xt_d.ap()[:, s * 2048:(s + 1) * 2048].rearrange(
                            "(c p) r -> p c r", p=P))
                    xts_list.append(xts)
                # adjT group: [128 j-part, 8 chunks, 1024 rows], i32 -> bf16;
                # two row-halves so stage B can start on the first half
                atg = atgp.tile([P, NST, RPC], BF16, tag="atg", name="atg")
                for hh in range(2):
                    rs = slice(hh * 512, (hh + 1) * 512)
                    nc.gpsimd.dma_start(
                        atg[:, :, rs],
                        adjt_d.ap()[s * 1024:(s + 1) * 1024, rs].rearrange(
                            "(t p) r -> p t r", p=P))
                atg_list.append(atg)

                if s < 4:
                    stage_a(s)
                scale_group(s)
                if s >= 1:
                    stage_b(s - 1)
            stage_b(NST - 1)

            # ---- alphas ----
            for mb in range(MB):
                m1 = sa.tile([P, 32], F32, tag="alm", name="alm1")
                nc.vector.tensor_mul(m1[:], aepm[:], selg[:, mb, :])
                nc.vector.tensor_reduce(al1[:, mb:mb + 1], m1[:], axis=AX,
                                        op=ADD)
                m2 = sa.tile([P, 32], F32, tag="alm", name="alm2")
                nc.vector.tensor_mul(m2[:], bepm[:], selg[:, mb, :])
                nc.vector.tensor_reduce(al2[:, mb:mb + 1], m2[:], axis=AX,
                                        op=ADD)
            nc.vector.tensor_scalar_add(al1[:], al1[:], gcol[:, 1:2])
            nc.vector.tensor_scalar_add(al2[:], al2[:], gcol[:, 1:2])

            # ---- combine + sigmoid + store ----
            for mb in range(MB):
                z1 = sa.tile([P, FOUT + 1], F32, tag="z1", name="z1")
                z2 = sa.tile([P, FOUT + 1], F32, tag="z2", name="z2")
                nc.vector.tensor_scalar_mul(z1[:], ysb[0][:, mb, :],
                                            al1[:, mb:mb + 1])
                nc.vector.tensor_scalar_mul(z2[:], ysb[1][:, mb, :],
                                            al2[:, mb:mb + 1])
                nc.vector.tensor_add(z1[:], z1[:], z2[:])
                rec = sa.tile([P, 1], F32, tag="rec", name="rec")
                nc.vector.reciprocal(rec[:], z1[:, FOUT:FOUT + 1])
                res = sa.tile([P, FOUT], F32, tag="res", name="res")
                nc.vector.tensor_scalar_mul(res[:], z1[:, :FOUT], rec[:])
                resg = sa.tile([P, FOUT], F32, tag="resg", name="resg")
                nc.scalar.activation(resg[:], res[:], Sigmoid)
                nc.sync.dma_start(y_d.ap()[mb * P:(mb + 1) * P, :], resg[:])

    nc.compile()
    return nc


_NC_CACHE = None


def _get_program():
    global _NC_CACHE
    if _NC_CACHE is None:
        _NC_CACHE = build_program()
    return _NC_CACHE


def _to_bf16(a):
    return np.ascontiguousarray(np.asarray(a, np.float32)).astype(
        ml_dtypes.bfloat16)


def make_in_maps(x, weight, att_vec, adj):
    x = np.asarray(x, dtype=np.float32)
    weight = np.ascontiguousarray(np.asarray(weight, dtype=np.float32))
    att_vec = np.asarray(att_vec, dtype=np.float32)
    adj = np.asarray(adj, dtype=np.int32)

    xt = _to_bf16(x.T)                             # [512, 8192] bf16
    attb = np.broadcast_to(att_vec[:, 0][None, :], (P, 2 * FOUT)).copy()
    adjT = np.ascontiguousarray(adj.T)             # [8192, 8192] int32

    emat = np.zeros((P, 4, P), np.float32)
    for i in range(P):
        if i < 64:
            emat[2 * i, 0, i] = 1
            emat[2 * i + 1, 2, i] = 1
        else:
            emat[2 * i - 128, 1, i] = 1
            emat[2 * i - 127, 3, i] = 1
    emat = _to_bf16(emat)

    in_maps = []
    for c in range(NCORES):
        g = 1.0 if c < 4 else 0.0
        gcol = np.empty((P, 2), np.float32)
        gcol[:, 0] = g
        gcol[:, 1] = 1.0 - g
        selg = np.zeros((P, MB, 32), np.float32)
        for mb in range(MB):
            selg[:, mb, (8 * c + mb) % 32] = g
        in_maps.append({
            "xt": xt,
            "w": weight,
            "attb": attb,
            "adjt": np.ascontiguousarray(adjT[:, c * RPC:(c + 1) * RPC]),
            "gcol": gcol,
            "selg": selg,
            "emat": emat,
        })
    return in_maps


def kernel(x, weight, att_vec, adj, _trace=False, _trace_kwargs=None):
    nc = _get_program()
    in_maps = make_in_maps(x, weight, att_vec, adj)
    r = run_bass_kernel_spmd(nc, in_maps, core_ids=list(range(NCORES)),
                             trace=_trace, **(_trace_kwargs or {}))
    y = np.concatenate([r.results[c]["y"] for c in range(NCORES)], axis=0)
    kernel.last_results = r
    return y.astype(np.float32)
